# revision 13
# baseline (speedup 1.0000x reference)
"""ClassAttention kernel for 8x TRN2 NeuronCores.

Reference computation (per batch element):
    qkv = x @ qkv_w.T + qkv_b                      # [N, 3C]
    q, k, v = split(qkv)                           # heads H=12, D=64
    s = softmax((q_cls . k) / sqrt(D))             # class-token query only
    cls = (s @ v) @ proj_w.T + proj_b              # [1, C]
    out = concat([cls, x[1:]])                     # rows 1..N pass through

Only the class token row changes, so the device computes just the [B, C]
cls output; rows 1..N are passed through on the host.

Sharding: data-parallel over batch, 8 batches per core, no collectives.

Algebraic structure (exploits the single class-token query; all device
matmuls are arranged so the matmul OUTPUT free dim is tiny -- the wide
operand is always the stationary one):
  - scores (transposed): sT[n, h] = sum_c xT[c, n] Wt[c, bh], with
    Wt = wk.T @ blockdiag(q + qb) computed once for all batches.  No k
    vector is ever materialized; k-bias cancels in softmax.
  - softmax: e = exp(sT) (scores are O(1): q.k/8 of unit-variance
    inputs, so no max-shift needed); den = ones.T @ e; e is normalized
    in place (e * (1/den) broadcast) so everything downstream is a
    plain linear map.
  - the v-projection commutes with the attention average:
    ZT[c, h] = x_b.T @ p = attention-average of x, per batch.  The
    x2 (token-major) operand tiles are produced ON CHIP by PE
    transposes of the single DMA'd xT layout -- x is read from HBM
    exactly once.
  - o (per head, all batches): oT[d, h, b] = wv_h.T @ ZT_b, 64-row
    psum blocks, no diagonal extraction needed.
  - proj (transposed): clsT[j, b] = sum_h wp64_h.T @ oT[:, h, :] with
    K=64 contraction chunks.  v-bias and proj bias fold into a host-side
    add: pb_eff = proj_b + vb @ proj_w.T (weight-only algebra).

Per-slot pipeline (slot k): A(k) scores | exp(k) | T(k) transposes
(+ evacuations rotated over DVE/ACT/Pool) | den/bcast/recip/mult(k) |
Z(k-1) | V(k-1), so every ACT/DVE round trip is covered by PE work and
the Z matmuls of a slot only consume fully-settled tiles.  The DMA
stream (x once + weights, ~12 MB) is issued in consumption order.
"""

import functools

import numpy as np
import ml_dtypes

import concourse.bass as bass
import concourse.tile as tile
from concourse import bacc, mybir, masks
from concourse import bass_utils

BF16 = mybir.dt.bfloat16
F32 = mybir.dt.float32
NPBF16 = ml_dtypes.bfloat16

B, N, C = 64, 577, 768
H, D = 12, 64
NCORES = 8
BPC = B // NCORES          # 8 batches per core
CT = C // 128              # 6 chunks of the feature dim
SCALE = D ** -0.5          # folded into wq/qb on the host

# token tiles: 5 chunks of <=128 (n on partitions after transpose)
T_TILES = [(0, 128), (128, 128), (256, 128), (384, 128), (512, 65)]
NTT = len(T_TILES)
C_HALVES = [(0, 512), (512, 256)]


def build_module():
    nc = bacc.Bacc("TRN2", target_bir_lowering=False, debug=False)

    xT_d = nc.dram_tensor("xT", [C, BPC, N], BF16, kind="ExternalInput")
    wq_d = nc.dram_tensor("wq", [C, C], BF16, kind="ExternalInput")    # [c, o]
    wk2_d = nc.dram_tensor("wk2", [C, C], BF16, kind="ExternalInput")  # [o, c]
    wv_d = nc.dram_tensor("wv", [C, C], BF16, kind="ExternalInput")    # [c, o]
    wp64_d = nc.dram_tensor("wp64", [D, H, C], BF16, kind="ExternalInput")
    xcls_d = nc.dram_tensor("xcls", [C, BPC], BF16, kind="ExternalInput")
    qb2_d = nc.dram_tensor("qb2", [128, CT], F32, kind="ExternalInput")
    clsT_d = nc.dram_tensor("clsT", [CT, 128, BPC], F32, kind="ExternalOutput")

    AF = mybir.ActivationFunctionType

    with tile.TileContext(nc) as tc:
        with (
            tc.tile_pool(name="const", bufs=1) as const,
            tc.tile_pool(name="xp", bufs=4) as xp,
            tc.tile_pool(name="x2p", bufs=3) as x2p,
            tc.tile_pool(name="sm", bufs=2) as sm,
            tc.tile_pool(name="ps", bufs=2, space="PSUM") as ps,
        ):
            # ---- DMAs, in the order the pipeline consumes them ----
            wq = const.tile([128, CT, C], BF16, tag="wq")
            nc.sync.dma_start(
                wq[:], wq_d.ap().rearrange("(a p) o -> p a o", p=128))
            wk2 = const.tile([128, CT, C], BF16, tag="wk2")
            nc.sync.dma_start(
                wk2[:], wk2_d.ap().rearrange("(a p) o -> p a o", p=128))
            xcls = const.tile([128, CT, BPC], BF16, tag="xcls")
            nc.sync.dma_start(
                xcls[:], xcls_d.ap().rearrange("(a p) b -> p a b", p=128))
            qb2 = const.tile([128, CT], F32, tag="qb2")
            nc.sync.dma_start(qb2[:], qb2_d.ap())

            xbs = {}

            def load_xb(b):
                xb = xp.tile([128, CT, N], BF16, tag="xb")
                nc.sync.dma_start(
                    xb[:],
                    xT_d.ap()[:, b, :].rearrange("(a p) t -> p a t", p=128))
                xbs[b] = xb

            for b in range(4):
                load_xb(b)
            wv = const.tile([128, CT, C], BF16, tag="wv")
            nc.sync.dma_start(
                wv[:], wv_d.ap().rearrange("(a p) o -> p a o", p=128))
            wp64 = const.tile([D, H, C], BF16, tag="wp64")

            # ---- constants ----
            identb = const.tile([128, 128], BF16, tag="identb")
            masks.make_identity(nc, identb[:])
            ones_n = const.tile([128, 1], BF16, tag="ones_n")
            nc.vector.memset(ones_n[:], 1.0)
            ones1 = const.tile([1, 128], BF16, tag="ones1")
            nc.vector.memset(ones1[:], 1.0)

            Qblk = const.tile([128, CT, BPC * H], BF16, tag="Qblk")
            nc.vector.memset(Qblk[:], 0.0)
            Wt = const.tile([128, CT, BPC * H], BF16, tag="Wt")
            ZT_all = const.tile([128, CT, BPC, H], BF16, tag="ZT_all")

            # ---- q for all 8 batches (transposed): qp[o, ci, b] ----
            qp = ps.tile([128, CT, BPC], F32, tag="sc")
            for ci in range(CT):
                for cj in range(CT):
                    nc.tensor.matmul(
                        qp[:, ci, :],
                        wq[:, cj, ci * 128:(ci + 1) * 128],
                        xcls[:, cj, :],
                        start=(cj == 0), stop=(cj == CT - 1))
            # Qblk[o, oj, b*12+h] = (q + qb)[b, o] for o in head h's block
            QblkV = Qblk[:].rearrange("p a (b h) -> p a b h", h=H)
            for oj in range(CT):
                for j in range(2):
                    h = 2 * oj + j
                    nc.vector.tensor_scalar_add(
                        QblkV[64 * j:64 * (j + 1), oj, :, h],
                        qp[64 * j:64 * (j + 1), oj, :],
                        qb2[64 * j:64 * (j + 1), oj:oj + 1])

            # ---- Wt[c, b*12+h] = wk.T @ Qblk ----
            for cj in range(CT):
                wtp = ps.tile([128, BPC * H], F32, tag="tp", bufs=3)
                for oj in range(CT):
                    nc.tensor.matmul(
                        wtp[:, :], wk2[:, oj, cj * 128:(cj + 1) * 128],
                        Qblk[:, oj, :],
                        start=(oj == 0), stop=(oj == CT - 1))
                nc.vector.tensor_copy(Wt[:, cj, :], wtp[:, :])

            # ---- per-batch software pipeline ----
            st = {}

            def emit_A(b):
                # transposed scores: sc[n, ti, h]
                xb = xbs[b]
                sc = ps.tile([128, NTT, H], F32, tag="sc")
                for ti, (to, tw) in enumerate(T_TILES):
                    for ci in range(CT):
                        nc.tensor.matmul(
                            sc[:tw, ti, :],
                            xb[:, ci, to:to + tw],
                            Wt[:, ci, b * H:(b + 1) * H],
                            start=(ci == 0), stop=(ci == CT - 1))
                st[b] = {"sc": sc}

            def emit_exp(b):
                s = st[b]
                e = sm.tile([128, NTT, H], BF16, tag="e")
                nc.scalar.activation(
                    e[:, 0:4, :].rearrange("p a h -> p (a h)"),
                    s["sc"][:, 0:4, :].rearrange("p a h -> p (a h)"),
                    AF.Exp, bias=0.0, scale=1.0)
                tw = T_TILES[-1][1]
                nc.scalar.activation(
                    e[:tw, 4, :], s["sc"][:tw, 4, :],
                    AF.Exp, bias=0.0, scale=1.0)
                s["e"] = e

            def emit_T(b, tis):
                # PE transposes xT -> x2 tiles (n on partitions)
                xb = xbs[b]
                s = st[b]
                x2t = s.get("x2t")
                if x2t is None:
                    x2t = x2p.tile([128, NTT, C], BF16, tag="x2t")
                    s["x2t"] = x2t
                for ti in tis:
                    to, tw = T_TILES[ti]
                    tp = ps.tile([128, CT, 128], BF16, tag="tp", bufs=3)
                    for ci in range(CT):
                        nc.tensor.transpose(
                            tp[:tw, ci, :], xb[:, ci, to:to + tw],
                            identb[:, :])
                    eng = s["engs"].pop(0)
                    eng(x2t[:tw, ti, :],
                        tp[:tw, :, :].rearrange("p a c -> p (a c)"))

            def emit_den(b):
                # den_row[1, h] = sum_n e[n, h]; bcast to rb[n, h]; recip
                s = st[b]
                e = s["e"]
                dn = ps.tile([1, H], F32, tag="dn", bufs=1)
                for ti, (to, tw) in enumerate(T_TILES):
                    nc.tensor.matmul(
                        dn[:, :], ones_n[:tw, :], e[:tw, ti, :],
                        start=(ti == 0), stop=(ti == NTT - 1))
                dsb = sm.tile([1, H], BF16, tag="dsb")
                nc.gpsimd.tensor_copy(dsb[:], dn[:])
                s["dsb"] = dsb

            def emit_bcast(b):
                s = st[b]
                rb = ps.tile([128, H], F32, tag="dn", bufs=1)
                nc.tensor.matmul(
                    rb[:, :], ones1[:, :], s["dsb"][:, :],
                    start=True, stop=True)
                s["rb"] = rb

            def emit_norm(b):
                # e_n = e * (1/den)  (broadcast over the 5 token tiles)
                s = st[b]
                rsb = sm.tile([128, H], F32, tag="rsb")
                nc.vector.reciprocal(rsb[:], s["rb"][:])
                e = s["e"]
                e_n = sm.tile([128, NTT, H], BF16, tag="e_n")
                rext = bass.AP(
                    rsb.tensor, rsb.offset,
                    [rsb.ap[0], [0, 4], rsb.ap[1]])
                nc.gpsimd.tensor_tensor(
                    e_n[:, 0:4, :], e[:, 0:4, :], rext, mybir.AluOpType.mult)
                tw = T_TILES[-1][1]
                nc.gpsimd.tensor_tensor(
                    e_n[:tw, 4, :], e[:tw, 4, :], rsb[:tw, :],
                    mybir.AluOpType.mult)
                s["e_n"] = e_n

            def emit_Z(b):
                # ZT[c, cj, h] = x_b.T @ p  (attention average, transposed)
                s = st.pop(b)
                del xbs[b]
                x2t, e_n = s["x2t"], s["e_n"]
                zt = ps.tile([128, CT, H], F32, tag="zt", bufs=1)
                for cj in range(CT):
                    for ti, (to, tw) in enumerate(T_TILES):
                        nc.tensor.matmul(
                            zt[:, cj, :],
                            x2t[:tw, ti, cj * 128:(cj + 1) * 128],
                            e_n[:tw, ti, :],
                            start=(ti == 0), stop=(ti == NTT - 1))
                nc.vector.tensor_copy(
                    ZT_all[:, :, b, :],
                    zt[:, :, :])

            def slot_engines():
                return [
                    lambda o, i: nc.vector.tensor_copy(o, i),
                    lambda o, i: nc.scalar.copy(o, i),
                    lambda o, i: nc.gpsimd.tensor_copy(o, i),
                    lambda o, i: nc.vector.tensor_copy(o, i),
                    lambda o, i: nc.vector.tensor_copy(o, i),
                ]

            # o per head: po[d, h, b], staged by batch group so most of
            # it runs inside the pipeline (off the critical path)
            po = ps.tile([D, H, BPC], F32, tag="po", bufs=1)
            oT = const.tile([D, H, BPC], BF16, tag="oT")

            def emit_O(b0, b1):
                for h in range(H):
                    for ci in range(CT):
                        nc.tensor.matmul(
                            po[:, h, b0:b1],
                            wv[:, ci, D * h:D * (h + 1)],
                            ZT_all[:, ci, b0:b1, h],
                            start=(ci == 0), stop=(ci == CT - 1))
                nc.vector.tensor_copy(oT[:, :, b0:b1], po[:, :, b0:b1])

            for k in range(BPC):
                if 1 <= k <= 4:
                    load_xb(k + 3)
                if k == 4:
                    nc.sync.dma_start(wp64[:], wp64_d.ap())
                if k > 0:
                    emit_Z(k - 1)
                if k == 5:
                    emit_O(0, 4)
                emit_A(k)
                st[k]["engs"] = slot_engines()
                emit_exp(k)
                emit_T(k, [0, 1, 2, 3])
                emit_den(k)
                emit_bcast(k)
                emit_T(k, [4])
                emit_norm(k)
            emit_Z(BPC - 1)
            emit_O(4, BPC)

            # ---- proj (transposed): clsT[j, cj, b] ----
            ct = ps.tile([128, CT, BPC], F32, tag="sc")
            for cj in range(CT):
                for h in range(H):
                    nc.tensor.matmul(
                        ct[:, cj, :],
                        wp64[:, h, cj * 128:(cj + 1) * 128],
                        oT[:, h, :],
                        start=(h == 0), stop=(h == H - 1))
            cls_sb = const.tile([128, CT, BPC], F32, tag="cls_sb")
            nc.vector.tensor_copy(cls_sb[:], ct[:])
            nc.sync.dma_start(
                clsT_d.ap().rearrange("a p b -> p a b"), cls_sb[:])

    nc.compile()
    return nc


@functools.lru_cache(maxsize=1)
def _module():
    return build_module()


def make_in_maps(x, qkv_w, qkv_b, proj_w, proj_b):
    x = np.asarray(x, dtype=np.float32)
    qkv_w = np.asarray(qkv_w, dtype=np.float32)
    qkv_b = np.asarray(qkv_b, dtype=np.float32)
    proj_w = np.asarray(proj_w, dtype=np.float32)
    proj_b = np.asarray(proj_b, dtype=np.float32)

    wq = np.ascontiguousarray(qkv_w[:C].T * SCALE).astype(NPBF16)   # [c, o]
    wk2 = np.ascontiguousarray(qkv_w[C:2 * C]).astype(NPBF16)       # [o, c]
    wv = np.ascontiguousarray(qkv_w[2 * C:].T).astype(NPBF16)       # [c, o]
    # wp64[d, h, j] = proj_w[j, 64h + d]
    wp64 = np.ascontiguousarray(
        proj_w.T.reshape(H, D, C).transpose(1, 0, 2)).astype(NPBF16)
    # q bias, pre-scaled, in [p, a] layout (o = a*128 + p)
    qbs = (qkv_b[:C] * SCALE).astype(np.float32)
    qb2 = np.ascontiguousarray(qbs.reshape(CT, 128).T)              # [128, 6]

    in_maps = []
    for i in range(NCORES):
        xs = x[i * BPC:(i + 1) * BPC]                               # [8, N, C]
        xT = np.ascontiguousarray(xs.transpose(2, 0, 1)).astype(NPBF16)
        xcls = np.ascontiguousarray(xs[:, 0, :].T).astype(NPBF16)   # [C, 8]
        in_maps.append({
            "xT": xT, "wq": wq, "wk2": wk2, "wv": wv, "wp64": wp64,
            "xcls": xcls, "qb2": qb2,
        })
    return in_maps


def kernel(x, qkv_w, qkv_b, proj_w, proj_b):
    nc = _module()
    in_maps = make_in_maps(x, qkv_w, qkv_b, proj_w, proj_b)
    res = bass_utils.run_bass_kernel_spmd(
        nc, in_maps, core_ids=list(range(NCORES)))
    # v bias contributes exactly (vb @ proj_w.T); fold into the proj bias
    # and add on the host (weight-only algebra).
    qkv_b = np.asarray(qkv_b, dtype=np.float32)
    pb_eff = np.asarray(proj_b, dtype=np.float32) + qkv_b[2 * C:] @ np.asarray(
        proj_w, dtype=np.float32).T
    out = np.array(np.asarray(x), dtype=np.float32, copy=True)
    for i in range(NCORES):
        clsT = np.asarray(res.results[i]["clsT"])                   # [6,128,8]
        cls = clsT.reshape(C, BPC).T + pb_eff                       # [8, C]
        out[i * BPC:(i + 1) * BPC, 0, :] = cls
    return out


# revision 14
# speedup vs baseline: 1.0604x; 1.0604x over previous
"""ClassAttention kernel for 8x TRN2 NeuronCores.

Reference computation (per batch element):
    qkv = x @ qkv_w.T + qkv_b                      # [N, 3C]
    q, k, v = split(qkv)                           # heads H=12, D=64
    s = softmax((q_cls . k) / sqrt(D))             # class-token query only
    cls = (s @ v) @ proj_w.T + proj_b              # [1, C]
    out = concat([cls, x[1:]])                     # rows 1..N pass through

Only the class token row changes, so the device computes just the [B, C]
cls output; rows 1..N are passed through on the host.

Sharding: data-parallel over batch, 8 batches per core, no collectives.

Algebraic structure (exploits the single class-token query; all device
matmuls are arranged so the matmul OUTPUT free dim is tiny -- the wide
operand is always the stationary one):
  - scores (transposed): sT[n, h] = sum_c xT[c, n] Wt[c, bh], with
    Wt = wk.T @ blockdiag(q + qb) computed once for all batches.  No k
    vector is ever materialized; k-bias cancels in softmax.
  - softmax: e = exp(sT) (scores are O(1): q.k/8 of unit-variance
    inputs, so no max-shift needed); den = ones.T @ e; e is normalized
    in place (e * (1/den) broadcast) so everything downstream is a
    plain linear map.
  - the v-projection commutes with the attention average:
    ZT[c, h] = x_b.T @ p = attention-average of x, per batch.  The
    x2 (token-major) operand tiles are produced ON CHIP by PE
    transposes of the single DMA'd xT layout -- x is read from HBM
    exactly once.
  - o (per head, all batches): oT[d, h, b] = wv_h.T @ ZT_b, 64-row
    psum blocks, no diagonal extraction needed.
  - proj (transposed): clsT[j, b] = sum_h wp64_h.T @ oT[:, h, :] with
    K=64 contraction chunks.  v-bias and proj bias fold into a host-side
    add: pb_eff = proj_b + vb @ proj_w.T (weight-only algebra).

Per-slot pipeline (slot k): A(k) scores | exp(k) | T(k) transposes
(+ evacuations rotated over DVE/ACT/Pool) | den/bcast/recip/mult(k) |
Z(k-1) | V(k-1), so every ACT/DVE round trip is covered by PE work and
the Z matmuls of a slot only consume fully-settled tiles.  The DMA
stream (x once + weights, ~12 MB) is issued in consumption order.
"""

import functools

import numpy as np
import ml_dtypes

import concourse.bass as bass
import concourse.tile as tile
from concourse import bacc, mybir, masks
from concourse import bass_utils

BF16 = mybir.dt.bfloat16
F32 = mybir.dt.float32
NPBF16 = ml_dtypes.bfloat16

B, N, C = 64, 577, 768
H, D = 12, 64
NCORES = 8
BPC = B // NCORES          # 8 batches per core
CT = C // 128              # 6 chunks of the feature dim
SCALE = D ** -0.5          # folded into wq/qb on the host

# token tiles: 5 chunks of <=128 (n on partitions after transpose)
T_TILES = [(0, 128), (128, 128), (256, 128), (384, 128), (512, 65)]
NTT = len(T_TILES)
C_HALVES = [(0, 512), (512, 256)]


def build_module():
    nc = bacc.Bacc("TRN2", target_bir_lowering=False, debug=False)

    xT_d = nc.dram_tensor("xT", [C, BPC, N], BF16, kind="ExternalInput")
    wq_d = nc.dram_tensor("wq", [C, C], BF16, kind="ExternalInput")    # [c, o]
    wk2_d = nc.dram_tensor("wk2", [C, C], BF16, kind="ExternalInput")  # [o, c]
    wv_d = nc.dram_tensor("wv", [C, C], BF16, kind="ExternalInput")    # [c, o]
    wp64_d = nc.dram_tensor("wp64", [D, H, C], BF16, kind="ExternalInput")
    xcls_d = nc.dram_tensor("xcls", [C, BPC], BF16, kind="ExternalInput")
    qb2_d = nc.dram_tensor("qb2", [128, CT], F32, kind="ExternalInput")
    clsT_d = nc.dram_tensor("clsT", [CT, 128, BPC], F32, kind="ExternalOutput")

    AF = mybir.ActivationFunctionType

    with tile.TileContext(nc) as tc:
        with (
            tc.tile_pool(name="const", bufs=1) as const,
            tc.tile_pool(name="xp", bufs=4) as xp,
            tc.tile_pool(name="x2p", bufs=3) as x2p,
            tc.tile_pool(name="sm", bufs=2) as sm,
            tc.tile_pool(name="ps", bufs=2, space="PSUM") as ps,
        ):
            # ---- DMAs, in the order the pipeline consumes them ----
            wq = const.tile([128, CT, C], BF16, tag="wq")
            nc.sync.dma_start(
                wq[:], wq_d.ap().rearrange("(a p) o -> p a o", p=128))
            wk2 = const.tile([128, CT, C], BF16, tag="wk2")
            nc.sync.dma_start(
                wk2[:], wk2_d.ap().rearrange("(a p) o -> p a o", p=128))
            xcls = const.tile([128, CT, BPC], BF16, tag="xcls")
            nc.sync.dma_start(
                xcls[:], xcls_d.ap().rearrange("(a p) b -> p a b", p=128))
            qb2 = const.tile([128, CT], F32, tag="qb2")
            nc.sync.dma_start(qb2[:], qb2_d.ap())

            xbs = {}

            def load_xb(b):
                xb = xp.tile([128, CT, N], BF16, tag="xb")
                nc.sync.dma_start(
                    xb[:],
                    xT_d.ap()[:, b, :].rearrange("(a p) t -> p a t", p=128))
                xbs[b] = xb

            for b in range(4):
                load_xb(b)
            wv = const.tile([128, CT, C], BF16, tag="wv")
            nc.sync.dma_start(
                wv[:], wv_d.ap().rearrange("(a p) o -> p a o", p=128))
            wp64 = const.tile([D, H, C], BF16, tag="wp64")

            # ---- constants ----
            identb = const.tile([128, 128], BF16, tag="identb")
            masks.make_identity(nc, identb[:])
            ones_n = const.tile([128, 1], BF16, tag="ones_n")
            nc.vector.memset(ones_n[:], 1.0)
            ones1 = const.tile([1, 128], BF16, tag="ones1")
            nc.vector.memset(ones1[:], 1.0)

            Qblk = const.tile([128, CT, BPC * H], BF16, tag="Qblk")
            nc.vector.memset(Qblk[:], 0.0)
            Wt = const.tile([128, CT, BPC * H], BF16, tag="Wt")
            ZT_all = const.tile([128, CT, BPC, H], BF16, tag="ZT_all")

            # ---- q for all 8 batches (transposed): qp[o, ci, b] ----
            qp = ps.tile([128, CT, BPC], F32, tag="sc")
            for ci in range(CT):
                for cj in range(CT):
                    nc.tensor.matmul(
                        qp[:, ci, :],
                        wq[:, cj, ci * 128:(ci + 1) * 128],
                        xcls[:, cj, :],
                        start=(cj == 0), stop=(cj == CT - 1))
            # Qblk[o, oj, b*12+h] = (q + qb)[b, o] for o in head h's block
            QblkV = Qblk[:].rearrange("p a (b h) -> p a b h", h=H)
            for oj in range(CT):
                for j in range(2):
                    h = 2 * oj + j
                    nc.vector.tensor_scalar_add(
                        QblkV[64 * j:64 * (j + 1), oj, :, h],
                        qp[64 * j:64 * (j + 1), oj, :],
                        qb2[64 * j:64 * (j + 1), oj:oj + 1])

            # ---- Wt[c, b*12+h] = wk.T @ Qblk ----
            for cj in range(CT):
                wtp = ps.tile([128, BPC * H], F32, tag="tp", bufs=3)
                for oj in range(CT):
                    nc.tensor.matmul(
                        wtp[:, :], wk2[:, oj, cj * 128:(cj + 1) * 128],
                        Qblk[:, oj, :],
                        start=(oj == 0), stop=(oj == CT - 1))
                nc.vector.tensor_copy(Wt[:, cj, :], wtp[:, :])

            # ---- per-batch software pipeline ----
            st = {}

            def emit_A(b):
                # transposed scores: sc[n, ti, h]
                xb = xbs[b]
                sc = ps.tile([128, NTT, H], F32, tag="sc")
                for ti, (to, tw) in enumerate(T_TILES):
                    for ci in range(CT):
                        nc.tensor.matmul(
                            sc[:tw, ti, :],
                            xb[:, ci, to:to + tw],
                            Wt[:, ci, b * H:(b + 1) * H],
                            start=(ci == 0), stop=(ci == CT - 1))
                st[b] = {"sc": sc}

            def emit_exp(b):
                s = st[b]
                e = sm.tile([128, NTT, H], BF16, tag="e")
                nc.scalar.activation(
                    e[:, 0:4, :].rearrange("p a h -> p (a h)"),
                    s["sc"][:, 0:4, :].rearrange("p a h -> p (a h)"),
                    AF.Exp, bias=0.0, scale=1.0)
                tw = T_TILES[-1][1]
                nc.scalar.activation(
                    e[:tw, 4, :], s["sc"][:tw, 4, :],
                    AF.Exp, bias=0.0, scale=1.0)
                s["e"] = e

            def emit_T(b, tis):
                # PE transposes xT -> x2 tiles (n on partitions)
                xb = xbs[b]
                s = st[b]
                x2t = s.get("x2t")
                if x2t is None:
                    x2t = x2p.tile([128, NTT, C], BF16, tag="x2t")
                    s["x2t"] = x2t
                for ti in tis:
                    to, tw = T_TILES[ti]
                    tp = ps.tile([128, CT, 128], BF16, tag="tp", bufs=3)
                    for ci in range(CT):
                        nc.tensor.transpose(
                            tp[:tw, ci, :], xb[:, ci, to:to + tw],
                            identb[:, :])
                    eng = s["engs"].pop(0)
                    eng(x2t[:tw, ti, :],
                        tp[:tw, :, :].rearrange("p a c -> p (a c)"))

            def emit_den(b):
                # den_row[1, h] = sum_n e[n, h]; bcast to rb[n, h]; recip
                s = st[b]
                e = s["e"]
                dn = ps.tile([1, H], F32, tag="dn", bufs=1)
                for ti, (to, tw) in enumerate(T_TILES):
                    nc.tensor.matmul(
                        dn[:, :], ones_n[:tw, :], e[:tw, ti, :],
                        start=(ti == 0), stop=(ti == NTT - 1))
                dsb = sm.tile([1, H], BF16, tag="dsb")
                nc.vector.tensor_copy(dsb[:], dn[:])
                s["dsb"] = dsb

            def emit_bcast(b):
                s = st[b]
                rb = ps.tile([128, H], F32, tag="dn", bufs=1)
                nc.tensor.matmul(
                    rb[:, :], ones1[:, :], s["dsb"][:, :],
                    start=True, stop=True)
                s["rb"] = rb

            def emit_norm(b):
                # e_n = e * (1/den)  (broadcast over the 5 token tiles)
                s = st[b]
                rsb = sm.tile([128, H], F32, tag="rsb")
                nc.vector.reciprocal(rsb[:], s["rb"][:])
                e = s["e"]
                e_n = sm.tile([128, NTT, H], BF16, tag="e_n")
                rext = bass.AP(
                    rsb.tensor, rsb.offset,
                    [rsb.ap[0], [0, 4], rsb.ap[1]])
                nc.vector.tensor_tensor(
                    e_n[:, 0:4, :], e[:, 0:4, :], rext, mybir.AluOpType.mult)
                tw = T_TILES[-1][1]
                nc.vector.tensor_tensor(
                    e_n[:tw, 4, :], e[:tw, 4, :], rsb[:tw, :],
                    mybir.AluOpType.mult)
                s["e_n"] = e_n

            def emit_Z(b):
                # ZT[c, cj, h] = x_b.T @ p  (attention average, transposed)
                s = st.pop(b)
                del xbs[b]
                x2t, e_n = s["x2t"], s["e_n"]
                zt = ps.tile([128, CT, H], F32, tag="zt", bufs=1)
                for cj in range(CT):
                    for ti, (to, tw) in enumerate(T_TILES):
                        nc.tensor.matmul(
                            zt[:, cj, :],
                            x2t[:tw, ti, cj * 128:(cj + 1) * 128],
                            e_n[:tw, ti, :],
                            start=(ti == 0), stop=(ti == NTT - 1))
                nc.vector.tensor_copy(
                    ZT_all[:, :, b, :],
                    zt[:, :, :])

            def slot_engines():
                return [
                    lambda o, i: nc.vector.tensor_copy(o, i),
                    lambda o, i: nc.scalar.copy(o, i),
                    lambda o, i: nc.gpsimd.tensor_copy(o, i),
                    lambda o, i: nc.vector.tensor_copy(o, i),
                    lambda o, i: nc.vector.tensor_copy(o, i),
                ]

            # o per head: po[d, h, b], staged by batch group so most of
            # it runs inside the pipeline (off the critical path)
            po = ps.tile([D, H, BPC], F32, tag="po", bufs=1)
            oT = const.tile([D, H, BPC], BF16, tag="oT")

            def emit_O(b0, b1):
                for h in range(H):
                    for ci in range(CT):
                        nc.tensor.matmul(
                            po[:, h, b0:b1],
                            wv[:, ci, D * h:D * (h + 1)],
                            ZT_all[:, ci, b0:b1, h],
                            start=(ci == 0), stop=(ci == CT - 1))
                nc.vector.tensor_copy(oT[:, :, b0:b1], po[:, :, b0:b1])

            for k in range(BPC):
                if 1 <= k <= 4:
                    load_xb(k + 3)
                if k == 4:
                    nc.sync.dma_start(wp64[:], wp64_d.ap())
                if k > 0:
                    emit_Z(k - 1)
                if k == 5:
                    emit_O(0, 4)
                emit_A(k)
                st[k]["engs"] = slot_engines()
                emit_exp(k)
                emit_T(k, [0, 1, 2, 3])
                emit_den(k)
                emit_bcast(k)
                emit_T(k, [4])
                emit_norm(k)
            emit_Z(BPC - 1)
            emit_O(4, BPC)

            # ---- proj (transposed): clsT[j, cj, b] ----
            ct = ps.tile([128, CT, BPC], F32, tag="sc")
            for cj in range(CT):
                for h in range(H):
                    nc.tensor.matmul(
                        ct[:, cj, :],
                        wp64[:, h, cj * 128:(cj + 1) * 128],
                        oT[:, h, :],
                        start=(h == 0), stop=(h == H - 1))
            cls_sb = const.tile([128, CT, BPC], F32, tag="cls_sb")
            nc.vector.tensor_copy(cls_sb[:], ct[:])
            nc.sync.dma_start(
                clsT_d.ap().rearrange("a p b -> p a b"), cls_sb[:])

    nc.compile()
    return nc


@functools.lru_cache(maxsize=1)
def _module():
    return build_module()


def make_in_maps(x, qkv_w, qkv_b, proj_w, proj_b):
    x = np.asarray(x, dtype=np.float32)
    qkv_w = np.asarray(qkv_w, dtype=np.float32)
    qkv_b = np.asarray(qkv_b, dtype=np.float32)
    proj_w = np.asarray(proj_w, dtype=np.float32)
    proj_b = np.asarray(proj_b, dtype=np.float32)

    wq = np.ascontiguousarray(qkv_w[:C].T * SCALE).astype(NPBF16)   # [c, o]
    wk2 = np.ascontiguousarray(qkv_w[C:2 * C]).astype(NPBF16)       # [o, c]
    wv = np.ascontiguousarray(qkv_w[2 * C:].T).astype(NPBF16)       # [c, o]
    # wp64[d, h, j] = proj_w[j, 64h + d]
    wp64 = np.ascontiguousarray(
        proj_w.T.reshape(H, D, C).transpose(1, 0, 2)).astype(NPBF16)
    # q bias, pre-scaled, in [p, a] layout (o = a*128 + p)
    qbs = (qkv_b[:C] * SCALE).astype(np.float32)
    qb2 = np.ascontiguousarray(qbs.reshape(CT, 128).T)              # [128, 6]

    in_maps = []
    for i in range(NCORES):
        xs = x[i * BPC:(i + 1) * BPC]                               # [8, N, C]
        xT = np.ascontiguousarray(xs.transpose(2, 0, 1)).astype(NPBF16)
        xcls = np.ascontiguousarray(xs[:, 0, :].T).astype(NPBF16)   # [C, 8]
        in_maps.append({
            "xT": xT, "wq": wq, "wk2": wk2, "wv": wv, "wp64": wp64,
            "xcls": xcls, "qb2": qb2,
        })
    return in_maps


def kernel(x, qkv_w, qkv_b, proj_w, proj_b):
    nc = _module()
    in_maps = make_in_maps(x, qkv_w, qkv_b, proj_w, proj_b)
    res = bass_utils.run_bass_kernel_spmd(
        nc, in_maps, core_ids=list(range(NCORES)))
    # v bias contributes exactly (vb @ proj_w.T); fold into the proj bias
    # and add on the host (weight-only algebra).
    qkv_b = np.asarray(qkv_b, dtype=np.float32)
    pb_eff = np.asarray(proj_b, dtype=np.float32) + qkv_b[2 * C:] @ np.asarray(
        proj_w, dtype=np.float32).T
    out = np.array(np.asarray(x), dtype=np.float32, copy=True)
    for i in range(NCORES):
        clsT = np.asarray(res.results[i]["clsT"])                   # [6,128,8]
        cls = clsT.reshape(C, BPC).T + pb_eff                       # [8, C]
        out[i * BPC:(i + 1) * BPC, 0, :] = cls
    return out
